# revision 3
# baseline (speedup 1.0000x reference)
"""Trainium2 Bass kernel for channel-wise spatial attention.

Reference computation (B=4, C=64, S=96, H=8):
  vqk = 1x1conv(x, w_vkq) + b_vkq            -> (B, 3*H*C, S, S)
  per (b,h,c):  score[r,t] = sum_y v[r,y]*k[t,y] / S^2 ; s = softmax_t
                out2[r,t]  = sum_y s[r,y]*q[t,y]
  out = 1x1conv(rearrange(out2, 'b h c x z -> b (c h) x z'), w_out) + b_out

Sharding: 8 cores = 4 batches x 2 head-halves (4 heads each). Each core
computes a partial to_out projection over its 256 (c,h) pairs; host sums
the two partials per batch and adds b_out.

Layout trick: the vkq projection uses the x-spatial-slice of the input as
the *stationary* matmul operand (lhsT = [x[b][:, xslice]; ones] of shape
[65, 96], bias folded in via the ones row), so projection outputs land as
[second-spatial-on-partitions, channel] tiles. That makes every attention
matmul transpose-free:
  mm1: score_T = K_slice^T @ V_slice           (psum [z, x])
  exp via ACT (scores ~1e-5, no max subtraction needed)
  mm2: out2 = E_T^T @ Q_slice, denominator via E_T^T @ ones column
  normalize with per-partition tensor_scalar multiply.
The (c,h) gather for to_out crosses the partition/free boundary via
SBUF->SBUF DMAs (one per channel) into HFIN[(h,c), pixel], then to_out is
K=128 PSUM-accumulated matmuls.
"""

import os
import sys
from contextlib import ExitStack

sys.path.insert(0, "/opt/trn_rl_repo")

import numpy as np

import concourse.bacc as bacc
import concourse.tile as tile
from concourse import mybir
from concourse.bass_utils import run_bass_kernel_spmd

B, C, S, H = 4, 64, 96, 8
NPIX = S * S
HL = H // 2      # heads per core
NQ = 8           # half-head groups per core
CL = 32          # attention channels per group
JW = 3 * CL      # projection channels per group (v,q,k)
NCORES = 8
XBATCH = 4       # x-slices per projection psum tile
FCH = 512        # final projection free-dim chunk

F32 = mybir.dt.float32
BF16 = mybir.dt.bfloat16

# compute dtype config: mmdt covers projection inputs + attention operands,
# findt covers the gathered out2 buffer + to_out weights.
CFG = {"mmdt": F32, "findt": F32}


def _body(ctx, tc, xe, wtg, w2t, outp, cfg):
    nc = tc.nc
    mmdt = cfg["mmdt"]
    findt = cfg["findt"]
    Exp = mybir.ActivationFunctionType.Exp

    const = ctx.enter_context(tc.tile_pool(name="const", bufs=1))
    projp = ctx.enter_context(tc.tile_pool(name="projp", bufs=1))
    obp = ctx.enter_context(tc.tile_pool(name="obp", bufs=2))
    etp = ctx.enter_context(tc.tile_pool(name="etp", bufs=3))
    rcp = ctx.enter_context(tc.tile_pool(name="rcp", bufs=3))
    stp = ctx.enter_context(tc.tile_pool(name="stp", bufs=3))
    pp_pool = ctx.enter_context(tc.tile_pool(name="pp", bufs=2, space="PSUM"))
    ps_pool = ctx.enter_context(tc.tile_pool(name="ps", bufs=2, space="PSUM"))
    po_pool = ctx.enter_context(tc.tile_pool(name="po", bufs=2, space="PSUM"))
    pf_pool = ctx.enter_context(tc.tile_pool(name="pf", bufs=2, space="PSUM"))

    XE = const.tile([C + 1, NPIX], mmdt)
    nc.sync.dma_start(XE[:], xe[:])
    WTG = const.tile([C + 1, NQ * JW], mmdt)
    nc.sync.dma_start(WTG[:], wtg[:])
    W2T = const.tile([128, 2 * C], findt)
    nc.sync.dma_start(W2T[:, 0:C], w2t[0:128, :])
    nc.sync.dma_start(W2T[:, C : 2 * C], w2t[128:256, :])
    ONES = const.tile([S, 1], mmdt)
    nc.vector.memset(ONES[:], 1.0)
    HFIN = const.tile([128, 2 * NPIX], findt)

    for q in range(NQ):
        # --- projection: PROJ[y, j*S + x] = vqk_raw[b, o(q,j), x, y] ---
        PROJ = projp.tile([S, JW * S], mmdt, tag="proj")
        projv = PROJ[:].rearrange("p (j x) -> p j x", x=S)
        for x0 in range(0, S, XBATCH):
            pp = pp_pool.tile([S, XBATCH * JW], F32, tag="pp")
            for i in range(XBATCH):
                nc.tensor.matmul(
                    pp[:, i * JW : (i + 1) * JW],
                    lhsT=XE[:, (x0 + i) * S : (x0 + i + 1) * S],
                    rhs=WTG[:, q * JW : (q + 1) * JW],
                    start=True,
                    stop=True,
                )
            nc.any.tensor_copy(
                projv[:, :, x0 : x0 + XBATCH],
                pp[:].rearrange("p (x j) -> p j x", j=JW),
            )

        # --- attention per channel ---
        OB = obp.tile([S, CL * S], findt, tag="ob")
        for cl in range(CL):
            vsl = PROJ[:, (0 * CL + cl) * S : (0 * CL + cl + 1) * S]
            qsl = PROJ[:, (1 * CL + cl) * S : (1 * CL + cl + 1) * S]
            ksl = PROJ[:, (2 * CL + cl) * S : (2 * CL + cl + 1) * S]
            ps = ps_pool.tile([S, S], F32, tag="ps")
            nc.tensor.matmul(ps[:], lhsT=ksl, rhs=vsl, start=True, stop=True)
            et = etp.tile([S, S], mmdt, tag="et")
            nc.scalar.activation(et[:], ps[:], Exp, scale=1.0 / NPIX)
            po = po_pool.tile([S, S + 1], F32, tag="po")
            nc.tensor.matmul(po[:, 0:S], lhsT=et[:], rhs=qsl, start=True, stop=True)
            nc.tensor.matmul(
                po[:, S : S + 1], lhsT=et[:], rhs=ONES[:], start=True, stop=True
            )
            rc = rcp.tile([S, 1], F32, tag="rc")
            nc.vector.reciprocal(rc[:], po[:, S : S + 1])
            nc.vector.tensor_scalar_mul(
                OB[:, cl * S : (cl + 1) * S], po[:, 0:S], rc[:]
            )

        # --- gather: OB[x, (cl, z)] -> HFIN[row, (x, z)] ---
        r0 = q * CL
        half, row = divmod(r0, 128)
        for cl in range(CL):
            nc.sync.dma_start(
                HFIN[row + cl : row + cl + 1, half * NPIX : (half + 1) * NPIX],
                OB[:, cl * S : (cl + 1) * S],
            )

    # --- to_out projection: contract all 256 (h,c) rows ---
    for n0 in range(0, NPIX, FCH):
        pf = pf_pool.tile([C, FCH], F32, tag="pf")
        nc.tensor.matmul(
            pf[:], lhsT=W2T[:, 0:C], rhs=HFIN[:, n0 : n0 + FCH], start=True, stop=False
        )
        nc.tensor.matmul(
            pf[:],
            lhsT=W2T[:, C : 2 * C],
            rhs=HFIN[:, NPIX + n0 : NPIX + n0 + FCH],
            start=False,
            stop=True,
        )
        fst = stp.tile([C, FCH], F32, tag="fst")
        nc.any.tensor_copy(fst[:], pf[:])
        nc.sync.dma_start(outp[:, n0 : n0 + FCH], fst[:])


_NC_CACHE = {}


def build_nc(cfg_key=None):
    cfg = CFG
    key = (cfg["mmdt"], cfg["findt"])
    if key in _NC_CACHE:
        return _NC_CACHE[key]
    npdt = mybir.dt.np(cfg["mmdt"])
    nc = bacc.Bacc("TRN2", target_bir_lowering=False, debug=False)
    xe = nc.dram_tensor("xe", [C + 1, NPIX], cfg["mmdt"], kind="ExternalInput").ap()
    wtg = nc.dram_tensor(
        "wtg", [C + 1, NQ * JW], cfg["mmdt"], kind="ExternalInput"
    ).ap()
    w2t = nc.dram_tensor("w2t", [2 * 128, C], cfg["findt"], kind="ExternalInput").ap()
    outp = nc.dram_tensor("outp", [C, NPIX], F32, kind="ExternalOutput").ap()
    with tile.TileContext(nc) as tc:
        with ExitStack() as ctx:
            _body(ctx, tc, xe, wtg, w2t, outp, cfg)
    nc.compile()
    _NC_CACHE[key] = nc
    return nc


def prep_in_maps(x, w_vkq, b_vkq, w_out, b_out):
    mmnp = np.dtype(mybir.dt.np(CFG["mmdt"]))
    finp = np.dtype(mybir.dt.np(CFG["findt"]))
    x = np.asarray(x, np.float32)
    w_vkq = np.asarray(w_vkq, np.float32)
    b_vkq = np.asarray(b_vkq, np.float32)
    w_out = np.asarray(w_out, np.float32)
    in_maps = []
    for core in range(NCORES):
        b, hh = divmod(core, 2)
        xe = np.concatenate(
            [x[b].reshape(C, NPIX), np.ones((1, NPIX), np.float32)], axis=0
        )
        wtg = np.empty((C + 1, NQ * JW), np.float32)
        w2t = np.empty((256, C), np.float32)
        for qq in range(NQ):
            h = hh * HL + qq // 2
            cb = (qq % 2) * CL
            for s in range(3):
                o = s * (H * C) + h * C + cb
                j = qq * JW + s * CL
                wtg[0:C, j : j + CL] = w_vkq[o : o + CL, :].T
                wtg[C, j : j + CL] = b_vkq[o : o + CL]
            for cl in range(CL):
                w2t[qq * CL + cl, :] = w_out[:, (cb + cl) * H + h]
        in_maps.append(
            {
                "xe": xe.astype(mmnp),
                "wtg": wtg.astype(mmnp),
                "w2t": w2t.astype(finp),
            }
        )
    return in_maps


def combine(results, b_out):
    b_out = np.asarray(b_out, np.float32)
    out = np.empty((B, C, S, S), np.float32)
    for b in range(B):
        part = results[2 * b]["outp"].astype(np.float32) + results[2 * b + 1][
            "outp"
        ].astype(np.float32)
        out[b] = part.reshape(C, S, S) + b_out[:, None, None]
    return out


def kernel(x, w_vkq, b_vkq, w_out, b_out):
    nc = build_nc()
    in_maps = prep_in_maps(x, w_vkq, b_vkq, w_out, b_out)
    r = run_bass_kernel_spmd(nc, in_maps, list(range(NCORES)), trace=False)
    kernel.last_result = r
    return combine(r.results, b_out)
